# revision 1
# baseline (speedup 1.0000x reference)
"""Masked dot-product attention on 8 Trainium2 NeuronCores.

Problem: q,k,v [64, 1024, 64] f32, valid_lens [64] int32.
  scores = q @ k^T / 8, mask keys >= valid_len to -1e6, softmax, @ v.

Strategy (per core: 8 batches, pure data parallelism, no collectives):
  - Host prep: pre-transpose q,k to [D, S] (contraction dim on partitions),
    pre-zero v rows past valid_len and append the 0/1 mask as a 65th column
    (vm).  The masked softmax denominator then falls out of the same matmul
    that computes attn @ v.  valid_len==0 batches reproduce the reference's
    uniform-softmax by zeroing q (scores==0) and unmasking all keys.
  - Device, per key-tile j: scoresT[j,q] = kT_tile.T.T @ qT with fp16
    operands (keys on partitions, fp32 PSUM accumulate; fp16 streams 1 PE
    cycle/row vs fp32's 4 and bf16's 8-bit mantissa would cost ~2e-3
    output error), exp on ScalarE (scale=1/8, bias=-3 bounds the fp16
    range; it cancels between numerator and denominator), written fp16.
    Scores for two key tiles go out as adjacent matmuls on disjoint PE
    row groups (K=64) so they can execute concurrently.
  - attn@v runs with the exp'd tile as the stationary operand:
    po[128q, 65] += expT_chunk.T.T @ [v|mask]_tile per 128-query chunk,
    fp32 PSUM.  The fp16 weights ride the fast weight-load path and only 65
    columns stream per chunk, and the result lands queries-on-partitions:
    the softmax division is then a cheap [128, 4] reciprocal plus
    per-partition tensor_scalar multiplies -- no transposes, no broadcasts.
  - Chunk accumulation groups sharing a PSUM bank run sequentially (a
    group's start clears has_written for the whole bank), so the qc loop is
    outer and all exp tiles of a batch stay resident in SBUF.
  - DMA dispatch is the hidden serial resource (~0.6us per dma_start per
    sequencer): one vm load and one output store per batch, inputs on the
    Sync queue, outputs on the GpSimd queue.
  - Per-batch key tiles are truncated to ceil(valid/128): masked tail tiles
    contribute exactly zero, so they are skipped.  Batches are rank-sorted
    by valid_len and dealt one per core per slot (same baked schedule on
    every core), shortest slots first so epilogues hide under later compute.
"""

import numpy as np

import concourse.bass as bass
import concourse.bacc as bacc
import concourse.tile as tile
from concourse import mybir
from concourse import bass_utils

B, S, D = 64, 1024, 64
NCORES = 8
NB = B // NCORES  # batch slots per core
P = 128
NJT = S // P  # max key tiles per batch
W = D + 1  # v columns + mask column
F32 = mybir.dt.float32
F16 = mybir.dt.float16

TRACE = False  # set by test harness to capture an NTFF profile
LAST_RESULTS = None  # BassKernelResults stash for the harness

_program_cache = {}


def _av_steps(nc, po_pool, osb_pool, rec_pool, out, s, jt, exs, vm_t):
    """Yield one emission step at a time: 8 attn@v chunk-groups, then the
    normalization epilogue.  The caller interleaves these steps between the
    NEXT batch's score/exp pairs so the PE queue alternates between feeding
    ScalarE (scores) and draining it (attn@v).

    Output accumulators: 8 query-chunks of [128, 65] (cols 0..63 =
    unnormalized out rows, col 64 = denominator); a 65-wide chunk can't
    cross a PSUM bank so they're split 4+4 over two banks.  One pending
    accumulation group per PSUM bank at a time: a group's start clears
    has_written for the whole bank, so the 4 chunk groups sharing a bank
    run sequentially (alternating banks lets two overlap).
    """
    po = [po_pool.tile([P, 4 * W], F32, tag=f"po{h}", name=f"po{h}")
          for h in range(2)]
    order = [0, 4, 1, 5, 2, 6, 3, 7]  # alternate banks
    for qc in order:
        dst = po[qc // 4]
        col = (qc % 4) * W
        for j in range(jt):
            nc.tensor.matmul(
                dst[:, col:col + W],
                lhsT=exs[j][:, qc * P:(qc + 1) * P],
                rhs=vm_t[:, j * W:(j + 1) * W],
                start=(j == 0), stop=(j == jt - 1),
            )
            # fine-grained steps: never queue more than ~4 attn@v matmuls
            # ahead of the next batch's scores, or ScalarE loses exp slots
            if j % 4 == 3:
                yield
        yield
    osb = osb_pool.tile([P, 8 * D], F32, tag="osb", name="osb")
    for h in range(2):
        po3 = po[h].rearrange("p (c w) -> p c w", w=W)
        recp = rec_pool.tile([P, 4], F32, tag="rec", name="recp")
        nc.vector.reciprocal(out=recp, in_=po3[:, :, D])
        for i in range(4):
            qc = 4 * h + i
            nc.vector.tensor_scalar_mul(
                osb[:, qc * D:(qc + 1) * D],
                po3[:, i, 0:D],
                recp[:, i:i + 1],
            )
    eng = nc.gpsimd if s % 2 == 0 else nc.sync
    eng.dma_start(
        out=out[s].rearrange("(c p) d -> p c d", p=P),
        in_=osb.rearrange("p (c d) -> p c d", d=D),
    )
    yield


def _build_program(jt_counts):
    nc = bacc.Bacc("TRN2", target_bir_lowering=False, debug=False,
                   num_devices=NCORES)
    qT = nc.dram_tensor("qT", [NB, D, S], F16, kind="ExternalInput").ap()
    kT = nc.dram_tensor("kT", [NB, D, S], F16, kind="ExternalInput").ap()
    vm = nc.dram_tensor("vm", [NB, S, W], F16, kind="ExternalInput").ap()
    out = nc.dram_tensor("out", [NB, S, D], F32, kind="ExternalOutput").ap()

    with tile.TileContext(nc) as tc:
        with (
            tc.tile_pool(name="singles", bufs=1) as singles,
            tc.tile_pool(name="qk", bufs=3) as qk_pool,
            tc.tile_pool(name="vmp", bufs=4) as vm_pool,
            tc.tile_pool(name="ex", bufs=2 * NJT + 2) as ex_pool,
            tc.tile_pool(name="osb", bufs=2) as osb_pool,
            tc.tile_pool(name="rec", bufs=4) as rec_pool,
            tc.tile_pool(name="ps_s", bufs=3, space="PSUM") as ps_pool,
            tc.tile_pool(name="ps_o", bufs=1, space="PSUM") as po_pool,
        ):
            # exp(s/8 - 3): the -3 bounds the fp16 exp range; it cancels
            # between numerator and denominator.
            bias_t = singles.tile([P, 1], F32)
            nc.vector.memset(bias_t, -3.0)

            pending = None  # unfinished attn@v/epilogue of previous batch
            drip = 1
            for s in range(NB):
                jt = jt_counts[s]
                # q/k replicated into both partition halves (0-stride DMA
                # source) so score matmuls for two key-tiles can run
                # concurrently on PE row-groups (0..63) and (64..127).
                qT_t = qk_pool.tile([2 * D, S], F16, tag="qT")
                kT_t = qk_pool.tile([2 * D, S], F16, tag="kT")
                nc.sync.dma_start(out=qT_t[0:D, :], in_=qT[s])
                nc.gpsimd.dma_start(out=qT_t[D:2 * D, :], in_=qT[s])
                nc.sync.dma_start(out=kT_t[0:D, 0:jt * P],
                                  in_=kT[s, :, 0:jt * P])
                nc.gpsimd.dma_start(out=kT_t[D:2 * D, 0:jt * P],
                                    in_=kT[s, :, 0:jt * P])
                # All key tiles of vm in one DMA: [128, jt*65], tile j at
                # columns [j*65, (j+1)*65).
                vm_t = vm_pool.tile([P, NJT * W], F16, tag="vm", name="vm_t")
                nc.sync.dma_start(
                    out=vm_t.rearrange("p (j w) -> p j w", w=W)[:, 0:jt, :],
                    in_=vm[s, 0:jt * P, :].rearrange("(j p) w -> p j w", p=P),
                )
                # Score matmuls go out in row-group-interleaved pairs --
                # adjacent PE-queue entries on disjoint row groups execute
                # concurrently, so a pair of key tiles costs one tile's time.
                exs = []
                for m in range(0, jt, 2):
                    js = list(range(m, min(m + 2, jt)))
                    pss = [ps_pool.tile([P, S], F32, tag="ps", name="ps")
                           for _ in js]
                    for half in range(2):
                        for r, j in enumerate(js):
                            nc.tensor.matmul(
                                pss[r][:, half * 512:(half + 1) * 512],
                                lhsT=kT_t[r * D:(r + 1) * D,
                                          j * P:(j + 1) * P],
                                rhs=qT_t[r * D:(r + 1) * D,
                                         half * 512:(half + 1) * 512],
                                start=True, stop=True,
                                tile_position=(r * D, 0),
                            )
                    for r, j in enumerate(js):
                        ex = ex_pool.tile([P, S], F16, tag="ex", name="ex")
                        nc.scalar.activation(
                            out=ex, in_=pss[r],
                            func=mybir.ActivationFunctionType.Exp,
                            scale=0.125, bias=bias_t)
                        exs.append(ex)
                        # drain a sliver of the previous batch's attn@v
                        # after each exp (keeps ScalarE and PE both fed),
                        # paced to finish just before this batch's own attn@v
                        if pending is not None:
                            for _ in range(drip):
                                if next(pending, "done") == "done":
                                    pending = None
                                    break
                if pending is not None:
                    for _ in pending:
                        pass
                pending = _av_steps(nc, po_pool, osb_pool, rec_pool, out,
                                    s, jt, exs, vm_t)
                nsteps = 8 * ((jt + 3) // 4) + 1
                nxt = jt_counts[s + 1] if s + 1 < NB else jt
                drip = max(1, -(-nsteps // max(nxt, 1))) + 1
            for _ in pending:
                pass
    nc.compile()
    return nc


def kernel(q, k, v, valid_lens):
    global LAST_RESULTS
    q = np.array(q, dtype=np.float32, copy=True)
    k = np.asarray(k, dtype=np.float32)
    v = np.asarray(v, dtype=np.float32)
    vl = np.asarray(valid_lens).astype(np.int64)

    # valid_len == 0: reference's softmax over an all-masked row is uniform.
    # Zeroed q gives scores == 0 -> exp == 1 over all (unmasked) keys: same.
    valid_eff = np.where(vl <= 0, S, np.minimum(vl, S))
    q[vl <= 0] = 0.0

    mask = (np.arange(S)[None, :] < valid_eff[:, None]).astype(np.float32)
    qT = np.ascontiguousarray(q.transpose(0, 2, 1)).astype(np.float16)
    kT = np.ascontiguousarray(k.transpose(0, 2, 1)).astype(np.float16)
    vm = np.concatenate([v * mask[:, :, None], mask[:, :, None]], axis=2)
    vm = np.ascontiguousarray(vm).astype(np.float16)

    # Rank-sort batches by effective length; slot s takes one batch of rank
    # group [8s, 8s+8) per core, so the baked per-slot tile count wastes
    # little work.  Shortest slots run first (see module docstring).
    order = np.argsort(-valid_eff, kind="stable")
    assign = order.reshape(NB, NCORES)[::-1]  # ascending tile counts
    jt_counts = tuple(
        int(np.ceil(valid_eff[assign[s]].max() / P)) for s in range(NB)
    )

    nc = _program_cache.get(jt_counts)
    if nc is None:
        nc = _build_program(jt_counts)
        _program_cache[jt_counts] = nc

    in_maps = []
    for c in range(NCORES):
        bs = assign[:, c]
        in_maps.append({
            "qT": np.ascontiguousarray(qT[bs]),
            "kT": np.ascontiguousarray(kT[bs]),
            "vm": np.ascontiguousarray(vm[bs]),
        })
    res = bass_utils.run_bass_kernel_spmd(
        nc, in_maps, core_ids=list(range(NCORES)), trace=TRACE,
    )
    LAST_RESULTS = res

    out = np.empty((B, S, D), dtype=np.float32)
    for c in range(NCORES):
        o = res.results[c]["out"]
        for s in range(NB):
            out[assign[s, c]] = o[s]
    return out



# revision 2
# speedup vs baseline: 1.4151x; 1.4151x over previous
"""Masked dot-product attention on 8 Trainium2 NeuronCores.

Problem: q,k,v [64, 1024, 64] f32, valid_lens [64] int32.
  scores = q @ k^T / 8, mask keys >= valid_len to -1e6, softmax, @ v.

Strategy (per core: 8 batches, pure data parallelism, no collectives):

  Host prep: batches rank-sorted by valid_len and dealt one per core per
  slot; per-batch key tiles truncated to jt = ceil(valid/128) (masked tail
  tiles contribute exactly zero).  Slot order is [smallest, largest, ...,
  2nd-smallest] so the pipeline fills fast and drains cheap.  All of a
  core's inputs are baked into ONE fp16 DRAM blob [128, W] (one dma_start
  per slot, ~0.4MB each): per slot a q block [128, 512] (rows 0-63 = q^T
  cols 0-511, rows 64-127 = q^T cols 512-1023 -- NO replication), then per
  key tile a k block [128, 128] (k^T tile in rows 0-63, replicated in rows
  64-127) and a vm block [128, 65] = [v*mask | mask] (keys on partitions).
  valid_len==0 batches reproduce the reference's uniform softmax by zeroed
  q + all-ones mask.

  Scores (PE): per pair of key tiles (j0, j1), 2 issue-slots of 2
  concurrent matmuls on disjoint PE row groups (K=64 each, fp16):
  slot A computes j0 x q-cols-lo and j1 x q-cols-hi, slot B the converse,
  so q streams once per key-tile pair with zero operand replication.
  Odd-tail tiles use one slot (lo/hi halves concurrently).

  exp (the softmax bottleneck, ~1.1us/tile of [128,1024]): split between
  TWO engines running in parallel.  ACT tiles: nc.scalar.activation Exp
  (scale=1/8, bias=beta) -> fp16.  DVE tiles: one-pass Schraudolph --
  i16 = f32_scores * (2^10/ln2)/8 + 13900, written through an int16
  bitcast of the fp16 tile; the int16 bit pattern IS ~exp(s/8 + beta) in
  fp16 (max ~3% sawtooth error; errors partly cancel through the shared
  matmul denominator).  beta = ln2*(13900-15316)/1024 matches the two
  paths; it cancels between numerator and denominator.  The +13900 offset
  keeps i16 positive down to s = -75 (9.4 sigma of the N(0,64) scores).
  Per-jt ACT/DVE tile split balances the two engines globally.

  attn@v (PE): po[128q, 65] += exp_tile.T.T @ [vm] per 128-query chunk,
  fp16 weights on the fast-weight-load path, fp32 PSUM.  Chunk groups
  alternate between two PSUM accumulator tiles so two accumulation groups
  overlap.  Emission is drip-fed between the NEXT batch's score/exp pairs
  so PE alternates between feeding ACT/DVE (scores) and draining them.

  Normalize (DVE): one reciprocal [128,4] + one broadcast tensor_tensor
  multiply [128,4,64] per po accumulator -> fp16 osb, stored per slot via
  one SWDGE dma_start on the otherwise-idle GpSimd queue.  Output DRAM is
  fp16 [slot, 128, 512] (chunk-major); the host transposes back to
  [1024, 64] f32.
"""

import numpy as np

import concourse.bass as bass
import concourse.bacc as bacc
import concourse.tile as tile
from concourse import mybir
from concourse import bass_utils

B, S, D = 64, 1024, 64
NCORES = 8
NB = B // NCORES  # batch slots per core
P = 128
NJT = S // P  # max key tiles per batch
W = D + 1  # v columns + mask column
F32 = mybir.dt.float32
F16 = mybir.dt.float16
I16 = mybir.dt.int16

# Schraudolph-fp16 exp constants (see module docstring).
A16 = (2.0 ** 10 / np.log(2.0)) / 8.0       # folds the 1/8 score scale
B16 = 13900.0
BETA = float(np.log(2.0) * (B16 - (15 * 1024 - 44)) / 1024.0)  # ~ -0.958

# ACT tile count per jt; the rest go to DVE (odd-indexed tiles first).
ACT_OF_JT = {1: 1, 2: 1, 3: 2, 4: 3, 5: 3, 6: 4, 7: 4, 8: 5}

TRACE = False  # set by test harness to capture an NTFF profile
LAST_RESULTS = None  # BassKernelResults stash for the harness

_program_cache = {}


def _dve_tiles(jt):
    """Tile indices assigned to the DVE (Schraudolph) exp path."""
    d = jt - ACT_OF_JT[jt]
    picked = [j for j in range(jt) if j % 2 == 1][:d]
    return set(picked)


def _slot_layout(jts):
    """Column offsets into the per-core input blob, per slot."""
    qofs, kofs, ofs = [], [], 0
    for jt in jts:
        qofs.append(ofs)
        kofs.append(ofs + 512)
        ofs += 512 + jt * (P + W)
    return qofs, kofs, ofs


def _av_steps(nc, po_pool, osb_pool, rec_pool, in_all, outb, t, jt, kofs, exs):
    """Yield one emission step at a time: 8 attn@v chunk-groups, the
    normalization, then the store.  The caller interleaves these steps
    between the NEXT batch's score/exp pairs."""
    po = [po_pool.tile([P, 4 * W], F32, tag=f"po{h}", name=f"po{h}")
          for h in range(2)]
    order = [0, 4, 1, 5, 2, 6, 3, 7]  # alternate accumulator tiles
    for qc in order:
        dst = po[qc // 4]
        col = (qc % 4) * W
        for j in range(jt):
            nc.tensor.matmul(
                dst[:, col:col + W],
                lhsT=exs[j][:, qc * P:(qc + 1) * P],
                rhs=in_all[:, kofs + j * (P + W) + P: kofs + (j + 1) * (P + W)],
                start=(j == 0), stop=(j == jt - 1),
            )
            if j % 4 == 3:
                yield
        yield
    osb = osb_pool.tile([P, 8 * D], F16, tag="osb", name="osb")
    osb3 = osb.rearrange("p (c d) -> p c d", d=D)
    for h in range(2):
        po3 = po[h].rearrange("p (c w) -> p c w", w=W)
        recp = rec_pool.tile([P, 4], F32, tag="rec", name="recp")
        nc.vector.reciprocal(out=recp, in_=po3[:, :, D])
        rb = recp.rearrange("p (c o) -> p c o", o=1).broadcast_to([P, 4, D])
        nc.vector.tensor_tensor(
            out=osb3[:, 4 * h:4 * h + 4, :],
            in0=po3[:, :, 0:D], in1=rb,
            op=mybir.AluOpType.mult,
        )
        yield
    nc.gpsimd.dma_start(out=outb[t], in_=osb)
    yield


def _build_program(jts):
    nc = bacc.Bacc("TRN2", target_bir_lowering=False, debug=False,
                   num_devices=NCORES)
    qofs, kofs, totw = _slot_layout(jts)
    inb = nc.dram_tensor("inb", [P, totw], F16, kind="ExternalInput").ap()
    outb = nc.dram_tensor("out", [NB, P, 8 * D], F16,
                          kind="ExternalOutput").ap()

    with tile.TileContext(nc) as tc:
        with (
            tc.tile_pool(name="singles", bufs=1) as singles,
            tc.tile_pool(name="ex", bufs=2 * NJT) as ex_pool,
            tc.tile_pool(name="osb", bufs=2) as osb_pool,
            tc.tile_pool(name="rec", bufs=4) as rec_pool,
            tc.tile_pool(name="ps_s", bufs=3, space="PSUM") as ps_pool,
            tc.tile_pool(name="ps_o", bufs=1, space="PSUM") as po_pool,
        ):
            in_all = singles.tile([P, totw], F16)
            bias_t = singles.tile([P, 1], F32)
            nc.vector.memset(bias_t, BETA)
            # All input loads upfront on the Sync HWDGE ring: FIFO delivery
            # in slot order stays ahead of compute.
            for t in range(NB):
                w = 512 + jts[t] * (P + W)
                nc.sync.dma_start(out=in_all[:, qofs[t]:qofs[t] + w],
                                  in_=inb[:, qofs[t]:qofs[t] + w])

            def kap(t, j, half):
                c = kofs[t] + j * (P + W)
                return in_all[half * D:(half + 1) * D, c:c + P]

            def qap(t, half):
                return in_all[half * D:(half + 1) * D,
                              qofs[t]:qofs[t] + 512]

            pending = None  # unfinished attn@v/epilogue of previous batch
            drip = 1
            for t in range(NB):
                jt = jts[t]
                dve = _dve_tiles(jt)
                exs = [None] * jt
                pss = [None] * jt
                for m in range(0, jt, 2):
                    if m + 1 < jt:
                        ps0 = ps_pool.tile([P, S], F32, tag="ps", name="ps")
                        ps1 = ps_pool.tile([P, S], F32, tag="ps", name="ps")
                        nc.tensor.matmul(ps0[:, 0:512], lhsT=kap(t, m, 0),
                                         rhs=qap(t, 0), start=True, stop=True,
                                         tile_position=(0, 0))
                        nc.tensor.matmul(ps1[:, 512:1024], lhsT=kap(t, m + 1, 1),
                                         rhs=qap(t, 1), start=True, stop=True,
                                         tile_position=(D, 0))
                        nc.tensor.matmul(ps1[:, 0:512], lhsT=kap(t, m + 1, 0),
                                         rhs=qap(t, 0), start=True, stop=True,
                                         tile_position=(0, 0))
                        nc.tensor.matmul(ps0[:, 512:1024], lhsT=kap(t, m, 1),
                                         rhs=qap(t, 1), start=True, stop=True,
                                         tile_position=(D, 0))
                        pss[m], pss[m + 1] = ps0, ps1
                        js = (m, m + 1)
                    else:
                        ps0 = ps_pool.tile([P, S], F32, tag="ps", name="ps")
                        nc.tensor.matmul(ps0[:, 0:512], lhsT=kap(t, m, 0),
                                         rhs=qap(t, 0), start=True, stop=True,
                                         tile_position=(0, 0))
                        nc.tensor.matmul(ps0[:, 512:1024], lhsT=kap(t, m, 1),
                                         rhs=qap(t, 1), start=True, stop=True,
                                         tile_position=(D, 0))
                        pss[m] = ps0
                        js = (m,)
                    for j in js:
                        ex = ex_pool.tile([P, S], F16, tag="ex", name="ex")
                        if j in dve:
                            nc.vector.tensor_scalar(
                                out=ex.bitcast(I16), in0=pss[j],
                                scalar1=float(A16), scalar2=float(B16),
                                op0=mybir.AluOpType.mult,
                                op1=mybir.AluOpType.add)
                        else:
                            nc.scalar.activation(
                                out=ex, in_=pss[j],
                                func=mybir.ActivationFunctionType.Exp,
                                scale=0.125, bias=bias_t)
                        exs[j] = ex
                        # drain a sliver of the previous batch's attn@v
                        # after each exp (keeps all engines fed)
                        if pending is not None:
                            for _ in range(drip):
                                if next(pending, "done") == "done":
                                    pending = None
                                    break
                if pending is not None:
                    for _ in pending:
                        pass
                pending = _av_steps(nc, po_pool, osb_pool, rec_pool, in_all,
                                    outb, t, jt, kofs[t], exs)
                nsteps = 8 * ((jt + 3) // 4) + 8 + 3
                nxt = jts[t + 1] if t + 1 < NB else jt
                drip = max(1, -(-nsteps // max(nxt, 1))) + 1
            for _ in pending:
                pass
    nc.compile()
    return nc


def kernel(q, k, v, valid_lens):
    global LAST_RESULTS
    q = np.array(q, dtype=np.float32, copy=True)
    k = np.asarray(k, dtype=np.float32)
    v = np.asarray(v, dtype=np.float32)
    vl = np.asarray(valid_lens).astype(np.int64)

    # valid_len == 0: reference's softmax over an all-masked row is uniform.
    # Zeroed q gives scores == 0 -> exp == const over all (unmasked) keys.
    valid_eff = np.where(vl <= 0, S, np.minimum(vl, S))
    q[vl <= 0] = 0.0

    mask = (np.arange(S)[None, :] < valid_eff[:, None]).astype(np.float32)
    qT = np.ascontiguousarray(q.transpose(0, 2, 1)).astype(np.float16)
    kT = np.ascontiguousarray(k.transpose(0, 2, 1)).astype(np.float16)
    vm = np.concatenate([v * mask[:, :, None], mask[:, :, None]], axis=2)
    vm = vm.astype(np.float16)

    # Rank-sort batches; slot s takes one batch of rank group [8s, 8s+8)
    # per core.  Schedule order: smallest first (fast fill), then largest
    # down to 2nd-smallest (cheap drain).
    order = np.argsort(-valid_eff, kind="stable")
    asc = order.reshape(NB, NCORES)[::-1]
    jts_asc = [int(np.ceil(valid_eff[asc[s]].max() / P)) for s in range(NB)]
    perm = [0] + list(range(NB - 1, 0, -1))
    assign = asc[perm]                      # [slot t, core c] -> batch
    jts = tuple(jts_asc[p] for p in perm)

    nc = _program_cache.get(jts)
    if nc is None:
        nc = _build_program(jts)
        _program_cache[jts] = nc

    qofs, kofs, totw = _slot_layout(jts)
    in_maps = []
    for c in range(NCORES):
        blob = np.zeros((P, totw), dtype=np.float16)
        for t in range(NB):
            b = assign[t, c]
            jt = jts[t]
            qb = blob[:, qofs[t]:qofs[t] + 512]
            qb[0:D] = qT[b][:, 0:512]
            qb[D:2 * D] = qT[b][:, 512:1024]
            for j in range(jt):
                c0 = kofs[t] + j * (P + W)
                blob[0:D, c0:c0 + P] = kT[b][:, j * P:(j + 1) * P]
                blob[D:2 * D, c0:c0 + P] = kT[b][:, j * P:(j + 1) * P]
                blob[:, c0 + P:c0 + P + W] = vm[b][j * P:(j + 1) * P, :]
        in_maps.append({"inb": blob})
    res = bass_utils.run_bass_kernel_spmd(
        nc, in_maps, core_ids=list(range(NCORES)), trace=TRACE,
    )
    LAST_RESULTS = res

    out = np.empty((B, S, D), dtype=np.float32)
    for c in range(NCORES):
        o = res.results[c]["out"]  # [NB, 128, 512] fp16
        for t in range(NB):
            out[assign[t, c]] = (
                o[t].reshape(P, 8, D).transpose(1, 0, 2)
                .reshape(S, D).astype(np.float32)
            )
    return out
